# revision 25
# baseline (speedup 1.0000x reference)
"""Trainium2 Bass kernel for segment_reduce (max over groups of min within group).

reference semantics:
    mins = min(x[:, groups], axis=-1)   # [B, G]
    out  = max(mins, axis=1, keepdims=True)  # [B, 1]

Strategy:
  - Pure data parallel: 8 NeuronCores, each handles B/8 = 131072 rows.
  - Host side: transpose x to feature-major [32, B] so each feature's column
    loads as a unit-stride SBUF tile [128, 1024] (rows spread over
    partition x free dims). DMA casts fp32 -> fp16 in flight (SWDGE).
  - Device side: the whole reduction is an elementwise min/max dataflow over
    32 feature tiles. The group structure is known at compile time (groups is
    tiny), so we bake an optimized op schedule:
      * drop dominated groups (superset feature-sets can never win the max)
      * factor via the distributive lattice law
            max_i min(f, A_i) = min(f, max_i A_i)
        building a 2-level trie (pivot f -> second pivot g -> max over third
        features), which cuts tensor ops to ~(G-1) + #f-nodes + #fg-nodes.
      * co-design the feature load order and the op order with a greedy
        dataflow simulation so compute starts with the first DMA chunk and
        stays fed while features stream in.
    All rewrites are exact in the (min,max) lattice. fp16 tensor_tensor runs
    in the DVE 2x perf mode; min/max are selections so the only rounding is
    the initial fp32->fp16 cast (rel err <= 2^-11).
  - Store: DMA casts fp16 -> fp32 on the way out.
"""

import os
from collections import Counter, OrderedDict

import numpy as np

_B, _N, _G = 1048576, 32, 64
_NC = 8
_R = _B // _NC  # rows per core
_P = 128
_W = _R // _P  # free-dim width per feature tile

# per-DMA feature counts: small first chunks so compute starts early.
# The first _NHEAD single-feature chunks are host-pre-cast to fp16 and loaded
# via HWDGE (no SWDGE Q7 spin-up on the critical path); the rest stream as
# fp32 with the SWDGE in-flight cast.
_NHEAD = 4
_CHUNKS = [2, 1, 1] + [2] * 14
assert sum(_CHUNKS) == _N

_DT_NAME = os.environ.get("KERNEL_DT", "float16")

_prog_cache: dict = {}


def _dedup(groups: np.ndarray):
    """Sorted-distinct tuples, dominated groups removed."""
    gs = [tuple(sorted(set(int(v) for v in g))) for g in groups]
    sets = [frozenset(g) for g in gs]
    kept = []
    for i, si in enumerate(sets):
        dominated = False
        for j, sj in enumerate(sets):
            if i == j:
                continue
            # a proper-subset group has min >= ours, so ours never wins the max
            if sj < si or (sj == si and j < i):
                dominated = True
                break
        if not dominated:
            kept.append(gs[i])
    return kept


def _greedy_assignment(todo):
    """Greedy 2-level factoring: returns [(f, g2) per group in todo order]."""
    idx = list(range(len(todo)))
    assign = [None] * len(todo)
    while idx:
        cnt = Counter(f for i in idx for f in todo[i])
        f = max(cnt, key=lambda k: (cnt[k], -k))
        cluster = [i for i in idx if f in todo[i]]
        idx = [i for i in idx if f not in todo[i]]
        rests = {i: tuple(v for v in todo[i] if v != f) for i in cluster}
        while rests:
            c2 = Counter(v for r in rests.values() for v in r)
            g2 = max(c2, key=lambda k: (c2[k], -k))
            for i in [i for i, r in rests.items() if g2 in r]:
                assign[i] = (f, g2)
                del rests[i]
            for i in [i for i, r in rests.items() if not r]:
                raise AssertionError("2-group rest empty before pick")
    return assign


def _search_assignment(todo, seed_assign, restarts=40, rng_seed=0):
    """Local search minimizing (#distinct f) + (#distinct (f,g2))."""
    import random

    rnd = random.Random(rng_seed)
    options = []
    for g in todo:
        if len(g) == 3:
            a, b, c = g
            options.append([(a, b), (a, c), (b, a), (b, c), (c, a), (c, b)])
        else:
            a, b = g
            options.append([(a, b), (b, a)])

    def cost(assign):
        fs = Counter()
        fgs = Counter()
        for f, g2 in assign:
            fs[f] += 1
            fgs[(f, g2)] += 1
        return len(fs) + len(fgs), fs, fgs

    def climb(assign):
        c, fs, fgs = cost(assign)

        def swap(cur, o):
            """Apply cur -> o, return cost delta."""
            d = 0
            fgs[cur] -= 1
            if fgs[cur] == 0:
                d -= 1
            fs[cur[0]] -= 1
            if fs[cur[0]] == 0:
                d -= 1
            if fgs.get(o, 0) == 0:
                d += 1
            fgs[o] = fgs.get(o, 0) + 1
            if fs.get(o[0], 0) == 0:
                d += 1
            fs[o[0]] = fs.get(o[0], 0) + 1
            return d

        improved = True
        while improved:
            improved = False
            for i, opts in enumerate(options):
                for o in opts:
                    cur = assign[i]
                    if o == cur:
                        continue
                    d = swap(cur, o)
                    if d < 0:
                        assign[i] = o
                        c += d
                        improved = True
                    else:
                        swap(o, cur)
        return c, assign

    best_c, best_a = climb(list(seed_assign))
    for _ in range(restarts):
        a0 = [rnd.choice(o) for o in options]
        c, a = climb(a0)
        if c < best_c:
            best_c, best_a = c, a
    return best_a


def _build_trie(kept):
    """2-level factoring via greedy + local search.

    Returns (trie, singles): trie is OrderedDict f -> OrderedDict(g -> [thirds])
    covering all groups of size >= 2; singles lists size-1 groups' features.
    """
    singles = [g[0] for g in kept if len(g) == 1]
    todo = [g for g in kept if len(g) >= 2]
    if not todo:
        return OrderedDict(), singles
    assign = _greedy_assignment(todo)
    assign = _search_assignment(todo, assign)
    trie = OrderedDict()
    for g, (f, g2) in zip(todo, assign):
        rem = [v for v in g if v != f and v != g2]
        third = rem[0] if rem else None
        trie.setdefault(f, OrderedDict()).setdefault(g2, []).append(third)
    return trie, singles


class _Sub:
    __slots__ = ("g2", "thirds_left", "chain", "value", "n_chain")

    def __init__(self, g2, thirds):
        self.g2 = g2
        self.thirds_left = list(thirds)  # real third features not yet consumed
        self.chain = None  # buf holding the running max of thirds
        self.n_chain = len(thirds)
        self.value = None  # final sub value: ("buf",i) or ("feat",g2)


class _Cluster:
    __slots__ = ("f", "subs", "vacc", "n_pending", "closed")

    def __init__(self, f, sub):
        self.f = f
        self.subs = []
        for g2, thirds in sub.items():
            real = [t for t in thirds if t is not None]
            if real and len(real) != len(thirds):
                raise AssertionError("dominated group survived dedup")
            self.subs.append(_Sub(g2, real))
        self.vacc = None  # running max over sub values
        self.n_pending = len(self.subs)  # subs not yet joined into vacc
        self.closed = False


def _coschedule(trie, singles):
    """Greedy co-design of feature load order and op order.

    Emits ops as soon as their operands' features are 'loaded'; when no op is
    ready, loads the feature that unlocks the most work. Returns
    (ops, forder, max_live_bufs).
    """
    clusters = [_Cluster(f, sub) for f, sub in trie.items()]
    singles_left = list(singles)

    ops = []
    nbuf = [0]
    live = [0]
    max_live = [0]

    def newbuf():
        nbuf[0] += 1
        live[0] += 1
        max_live[0] = max(max_live[0], live[0])
        return ("buf", nbuf[0])

    def freebuf(v):
        if v[0] == "buf":
            live[0] -= 1

    loaded = set()
    forder = []
    acc = ("acc",)
    acc_started = [False]

    def join_acc(v):
        if not acc_started[0]:
            if v[0] == "feat":
                ops.append(("copy", acc, v))
            else:
                # value is in a buf: fold the final op into acc instead when
                # possible -- handled by callers via direct dst. Fallback:
                ops.append(("copy", acc, v))
            acc_started[0] = True
        else:
            ops.append(("max", acc, acc, v))
        freebuf(v)

    def emit_one():
        """Emit one ready op if any. Returns True if emitted/progressed."""
        # 0. close a cluster (min with pivot + acc join)
        for c in clusters:
            if c.closed or c.n_pending or c.f not in loaded:
                continue
            v = c.vacc
            if v is None:
                raise AssertionError("cluster with no subs")
            if not acc_started[0]:
                ops.append(("min", acc, ("feat", c.f), v))
                acc_started[0] = True
                freebuf(v)
            elif v[0] == "buf":
                ops.append(("min", v, ("feat", c.f), v))
                ops.append(("max", acc, acc, v))
                freebuf(v)
            else:
                b = newbuf()
                ops.append(("min", b, ("feat", c.f), v))
                ops.append(("max", acc, acc, b))
                freebuf(b)
            c.closed = True
            return True
        # 1. finalize a sub (min with g2) / realize 2-group values
        for c in clusters:
            if c.closed:
                continue
            for s in c.subs:
                if s.value is not None:
                    continue
                if s.n_chain == 0:
                    if s.g2 in loaded:
                        s.value = ("feat", s.g2)
                        return True
                    continue
                if not s.thirds_left and s.chain is not None and s.g2 in loaded:
                    ops.append(("min", s.chain, ("feat", s.g2), s.chain))
                    s.value = s.chain
                    return True
        # 2. join a sub value into the cluster accumulator
        for c in clusters:
            if c.closed:
                continue
            for s in c.subs:
                if s.value is None or s.value[0] == "joined":
                    continue
                if c.vacc is None:
                    c.vacc = s.value
                    s.value = ("joined",)
                    c.n_pending -= 1
                    return True
                if c.vacc[0] == "buf":
                    ops.append(("max", c.vacc, c.vacc, s.value))
                    freebuf(s.value)
                elif s.value[0] == "buf":
                    ops.append(("max", s.value, c.vacc, s.value))
                    c.vacc = s.value
                else:
                    b = newbuf()
                    ops.append(("max", b, c.vacc, s.value))
                    c.vacc = b
                s.value = ("joined",)
                c.n_pending -= 1
                return True
        # 3. join a loaded single
        for i, f in enumerate(singles_left):
            if f in loaded and acc_started[0]:
                ops.append(("max", acc, acc, ("feat", f)))
                singles_left.pop(i)
                return True
            if f in loaded and not acc_started[0]:
                ops.append(("copy", acc, ("feat", f)))
                acc_started[0] = True
                singles_left.pop(i)
                return True
        # 4. extend a chain with a loaded third
        for c in clusters:
            if c.closed:
                continue
            for s in c.subs:
                if s.value is not None or s.n_chain == 0:
                    continue
                have = [t for t in s.thirds_left if t in loaded]
                if s.chain is None:
                    if len(s.thirds_left) == 1 and have:
                        # single third: chain is just the feature; the min
                        # with g2 consumes it directly
                        t = have[0]
                        if s.g2 in loaded:
                            b = newbuf()
                            ops.append(("min", b, ("feat", s.g2), ("feat", t)))
                            s.thirds_left.remove(t)
                            s.value = b
                            return True
                        continue
                    if len(have) >= 2:
                        b = newbuf()
                        ops.append(("max", b, ("feat", have[0]), ("feat", have[1])))
                        s.thirds_left.remove(have[0])
                        s.thirds_left.remove(have[1])
                        s.chain = b
                        return True
                    continue
                if have:
                    ops.append(("max", s.chain, s.chain, ("feat", have[0])))
                    s.thirds_left.remove(have[0])
                    return True
        return False

    def unlock_score(f):
        """How much work loading f unlocks right now (cheap heuristic)."""
        sc = 0
        for c in clusters:
            if c.closed:
                continue
            if f == c.f and c.n_pending == 0:
                sc += 30  # closes a cluster immediately
            for s in c.subs:
                if s.value is not None:
                    continue
                if f == s.g2:
                    if s.n_chain == 0:
                        sc += 6
                    elif not s.thirds_left and s.chain is not None:
                        sc += 20
                    elif len(s.thirds_left) == 1 and s.thirds_left[0] in loaded:
                        sc += 15
                    else:
                        sc += 2
                if f in s.thirds_left:
                    have = sum(1 for t in s.thirds_left if t in loaded)
                    if s.chain is not None or have >= 1:
                        sc += 10
                    else:
                        sc += 3
        for s1 in singles_left:
            if f == s1:
                sc += 8
        return sc

    all_feats = sorted(
        {c.f for c in clusters}
        | {s.g2 for c in clusters for s in c.subs}
        | {t for c in clusters for s in c.subs for t in s.thirds_left}
        | set(singles_left)
    )
    while True:
        if emit_one():
            continue
        remaining = [f for f in all_feats if f not in loaded]
        if not remaining:
            break
        best = max(remaining, key=lambda f: (unlock_score(f), -f))
        loaded.add(best)
        forder.append(best)

    # drain: everything loaded now; emit the rest
    while emit_one():
        pass
    assert not singles_left and all(c.closed for c in clusters), "incomplete"

    for f in range(_N):
        if f not in loaded:
            forder.append(f)
    n_tt = sum(1 for o in ops if o[0] in ("min", "max"))
    return ops, forder, n_tt, max_live[0]


def _simulate(ops, groups, x):
    """Numpy execution of the op list (for self-checks)."""
    bufs = {}

    def val(v):
        if v[0] == "feat":
            return x[:, v[1]]
        return bufs[v]

    for op in ops:
        if op[0] == "copy":
            bufs[op[1]] = val(op[2]).copy()
        elif op[0] == "min":
            bufs[op[1]] = np.minimum(val(op[2]), val(op[3]))
        else:
            bufs[op[1]] = np.maximum(val(op[2]), val(op[3]))
    return bufs[("acc",)]


def _make_plan(groups: np.ndarray):
    kept = _dedup(groups)
    trie, singles = _build_trie(kept)
    ops, forder, n_tt, max_live = _coschedule(trie, singles)
    # self-check the schedule against brute force on random data
    xs = np.random.default_rng(0).standard_normal((256, _N)).astype(np.float32)
    want = xs[:, np.asarray(groups)].min(-1).max(1)
    got = _simulate(ops, groups, xs)
    assert np.array_equal(got, want), "schedule self-check failed"
    stats = {
        "n_groups_kept": len(kept),
        "n_f_nodes": len(trie),
        "n_fg_nodes": sum(len(s) for s in trie.values()),
        "n_tt_ops": n_tt,
        "max_live_bufs": max_live,
    }
    return ops, forder, stats


def _build_program(groups_tuple, dt_name):
    from concourse import bacc, mybir
    from concourse.tile import TileContext

    groups = np.array(groups_tuple, dtype=np.int64)
    ops, forder, stats = _make_plan(groups)
    fpos = {f: i for i, f in enumerate(forder)}  # feature id -> storage slot

    DT = getattr(mybir.dt, dt_name)
    cast = dt_name != "float32"

    nc = bacc.Bacc("TRN2", debug=False, enable_asserts=False, num_devices=_NC)
    nhead = _NHEAD if cast else 0
    if nhead:
        xt16 = nc.dram_tensor("xt16", [nhead, _R], DT, kind="ExternalInput")
    xt = nc.dram_tensor("xt", [_N - nhead, _R], mybir.dt.float32, kind="ExternalInput")
    out = nc.dram_tensor("out", [_R], DT, kind="ExternalOutput")

    with TileContext(nc) as tc:
        with (
            tc.tile_pool(name="feats", bufs=1) as fpool,
            tc.tile_pool(name="work", bufs=max(8, stats["max_live_bufs"] + 2)) as wpool,
        ):
            chunk_tiles = []
            slot2chunk = {}
            s0 = 0
            for ci, csz in enumerate(_CHUNKS):
                t = fpool.tile([_P, csz, _W], DT, tag=f"chunk{ci}", name=f"chunk{ci}")
                if s0 < nhead:
                    # pre-cast head features: HWDGE, fires right after preamble
                    assert s0 + csz <= nhead
                    src = xt16[s0 : s0 + csz, :].rearrange("f (p w) -> p f w", p=_P)
                    nc.sync.dma_start(out=t[:], in_=src)
                else:
                    src = xt[s0 - nhead : s0 - nhead + csz, :].rearrange(
                        "f (p w) -> p f w", p=_P
                    )
                    # SWDGE casts fp32->fp16 in flight; HWDGE when no cast
                    dma = nc.gpsimd if cast else nc.sync
                    dma.dma_start(out=t[:], in_=src)
                chunk_tiles.append(t)
                for k in range(csz):
                    slot2chunk[s0 + k] = (ci, k)
                s0 += csz

            def feat_ap(f):
                ci, k = slot2chunk[fpos[f]]
                return chunk_tiles[ci][:, k, :]

            buf_tiles = {}
            acc_tile = wpool.tile([_P, _W], DT, tag="acc", name="acc")

            def val_ap(v):
                if v[0] == "feat":
                    return feat_ap(v[1])
                if v[0] == "acc":
                    return acc_tile[:]
                return buf_tiles[v][:]

            def dst_ap(v):
                if v[0] == "acc":
                    return acc_tile[:]
                if v not in buf_tiles:
                    buf_tiles[v] = wpool.tile(
                        [_P, _W], DT, tag="u", name=f"u{v[1]}"
                    )
                return buf_tiles[v][:]

            alu = {"min": mybir.AluOpType.min, "max": mybir.AluOpType.max}
            for op in ops:
                if op[0] == "copy":
                    nc.vector.tensor_copy(out=dst_ap(op[1]), in_=val_ap(op[2]))
                else:
                    nc.vector.tensor_tensor(
                        dst_ap(op[1]), val_ap(op[2]), val_ap(op[3]), alu[op[0]]
                    )

            nc.sync.dma_start(
                out=out.rearrange("(p w) -> p w", p=_P), in_=acc_tile[:]
            )

    nc.compile()
    return nc, forder, stats


def _build_program_raw(groups_tuple, dt_name):
    """Raw-Bacc variant: same op schedule, hand-rolled semaphores.

    The dataflow is simple (FIFO chunk loads -> in-order DVE ops -> store), so
    explicit sync needs just one DMA semaphore (SWDGE queue completes in issue
    order), a DVE-done semaphore, and a store-landed wait. This skips the Tile
    entry barrier and its end-of-kernel drain + event-semaphore butterfly.
    """
    from concourse import bacc, mybir

    groups = np.array(groups_tuple, dtype=np.int64)
    ops, forder, stats = _make_plan(groups)
    fpos = {f: i for i, f in enumerate(forder)}

    DT = getattr(mybir.dt, dt_name)
    cast = dt_name != "float32"

    # physical slot reuse for intermediate buffers (DVE executes in order, so
    # reuse after last read needs no synchronization)
    last_use = {}
    for i, op in enumerate(ops):
        for v in op[1:]:
            if isinstance(v, tuple) and v[0] == "buf":
                last_use[v] = i
    slot_of = {}
    free_slots = []
    nslots = 0
    for i, op in enumerate(ops):
        dst = op[1]
        if dst[0] == "buf" and dst not in slot_of:
            if free_slots:
                slot_of[dst] = free_slots.pop()
            else:
                slot_of[dst] = nslots
                nslots += 1
        for v in op[2:]:
            if (
                isinstance(v, tuple)
                and v[0] == "buf"
                and last_use.get(v) == i
                and v != dst
            ):
                free_slots.append(slot_of[v])
    stats = dict(stats, n_slots=nslots)

    nc = bacc.Bacc("TRN2", debug=False, enable_asserts=False, num_devices=_NC)
    xt = nc.dram_tensor("xt", [_N, _R], mybir.dt.float32, kind="ExternalInput")
    out = nc.dram_tensor("out", [_R], DT, kind="ExternalOutput")

    chunk_tiles = []
    for ci, csz in enumerate(_CHUNKS):
        chunk_tiles.append(nc.alloc_sbuf_tensor(f"chunk{ci}", [_P, csz, _W], DT))
    ubufs = nc.alloc_sbuf_tensor("ubufs", [_P, max(nslots, 1), _W], DT)
    acc_t = nc.alloc_sbuf_tensor("acc", [_P, _W], DT)

    slot2chunk = {}
    s0 = 0
    for ci, csz in enumerate(_CHUNKS):
        for k in range(csz):
            slot2chunk[s0 + k] = (ci, k)
        s0 += csz

    def feat_ap(f):
        ci, k = slot2chunk[fpos[f]]
        return chunk_tiles[ci].ap()[:, k, :]

    def val_ap(v):
        if v[0] == "feat":
            return feat_ap(v[1])
        if v[0] == "acc":
            return acc_t.ap()[:, :]
        return ubufs.ap()[:, slot_of[v], :]

    alu = {"min": mybir.AluOpType.min, "max": mybir.AluOpType.max}
    n_chunks = len(_CHUNKS)

    from contextlib import ExitStack

    with ExitStack() as ctx:
        # one semaphore per chunk DMA: a shared counter is racy (the 16 SDMA
        # engines' incs for successive DMAs interleave, so sem>=16(k+1) does
        # not imply chunk k landed)
        chunk_sems = [
            ctx.enter_context(nc.semaphore(name=f"ck_sem{ci}"))
            for ci in range(len(_CHUNKS))
        ]
        done_sem = ctx.enter_context(nc.semaphore())
        st_sem = ctx.enter_context(nc.semaphore())
        block = ctx.enter_context(nc.Block(no_gpsimd_drain=True))

        @block.gpsimd
        def _(g):
            s = 0
            for ci, csz in enumerate(_CHUNKS):
                src = xt[s : s + csz, :].rearrange("f (p w) -> p f w", p=_P)
                dma = g.dma_start(out=chunk_tiles[ci].ap()[:], in_=src)
                dma.then_inc(chunk_sems[ci], 16)
                s += csz

        @block.vector
        def _(v):
            waited = set()
            last = None
            for op in ops:
                for val in op[2:]:
                    if isinstance(val, tuple) and val[0] == "feat":
                        ci = slot2chunk[fpos[val[1]]][0]
                        if ci not in waited:
                            waited.add(ci)
                            v.wait_ge(chunk_sems[ci], 16)
                if op[0] == "copy":
                    last = nc.vector.tensor_copy(out=val_ap(op[1]), in_=val_ap(op[2]))
                else:
                    last = nc.vector.tensor_tensor(
                        val_ap(op[1]), val_ap(op[2]), val_ap(op[3]), alu[op[0]]
                    )
            last.then_inc(done_sem, 1)

        @block.sync
        def _(s):
            # all loads landed (covers chunks no op waited on) + compute done
            for ci in range(n_chunks):
                s.wait_ge(chunk_sems[ci], 16)
            s.wait_ge(done_sem, 1)
            st = s.dma_start(
                out=out.rearrange("(p w) -> p w", p=_P), in_=acc_t.ap()[:, :]
            )
            st.then_inc(st_sem, 16)
            s.wait_ge(st_sem, 16)

    nc.compile()
    return nc, forder, stats


def _get_program(groups: np.ndarray):
    impl = os.environ.get("KERNEL_IMPL", "tile")
    key = (tuple(map(tuple, np.asarray(groups).tolist())), _DT_NAME, impl)
    if key not in _prog_cache:
        builder = _build_program_raw if impl == "raw" else _build_program
        _prog_cache[key] = builder(key[0], _DT_NAME)
    return _prog_cache[key]


def run(x, groups, trace=False):
    """Returns (out [B,1] fp32, BassKernelResults, stats)."""
    from concourse import bass_utils

    x = np.asarray(x)
    groups = np.asarray(groups)
    assert x.shape == (_B, _N), x.shape
    nc, forder, stats = _get_program(groups)

    # feature-major, permuted into load order
    xt = np.ascontiguousarray(x.T[forder].astype(np.float32, copy=False))
    impl = os.environ.get("KERNEL_IMPL", "tile")
    nhead = _NHEAD if (_DT_NAME != "float32" and impl == "tile") else 0
    in_maps = []
    for c in range(_NC):
        sl = xt[:, c * _R : (c + 1) * _R]
        m = {"xt": np.ascontiguousarray(sl[nhead:])}
        if nhead:
            m["xt16"] = np.ascontiguousarray(sl[:nhead]).astype(np.float16)
        in_maps.append(m)
    res = bass_utils.run_bass_kernel_spmd(
        nc, in_maps, core_ids=list(range(_NC)), trace=trace
    )
    y = (
        np.concatenate([np.asarray(res.results[c]["out"]) for c in range(_NC)])
        .astype(np.float32, copy=False)
        .reshape(_B, 1)
    )
    return y, res, stats


def kernel(x, groups):
    y, _res, _stats = run(x, groups, trace=False)
    return y
